# revision 1
# baseline (speedup 1.0000x reference)
"""Grouped categorical log-softmax (segment logsumexp) on 8 Trainium2 cores.

Strategy: the index is sorted, so each segment is a contiguous run. On the host
we bucket segments by length (exact lengths 2..24, coarser canonical lengths for
the rare tail, padding inside a slot with -80 so exp() contributes nothing to
fp32 sums), shard every bucket evenly across the 8 cores, and lay each core's
data out as a dense [128, W_total] matrix where every bucket occupies a
contiguous block of columns holding 128*q fixed-length segment slots.

The device kernel is then a pure batched row-block log-softmax with static
shapes: exp (ScalarE) -> per-slot reduce_sum (VectorE) -> ln (ScalarE) ->
broadcast subtract (VectorE), streamed in ~2k-column groups overlapped with
HBM loads/stores. out = x - log(sum(exp(x))) is mathematically identical to
the reference's max-normalized form, and with standard-normal logits fp32
exp/log are nowhere near overflow, so skipping the max pass is numerically
safe (measured absmax error ~1e-5 against the fp32 reference).

Length-1 segments are exactly 0 in the reference, so they are filled on the
host. Empty segments produce no output elements.
"""
from contextlib import ExitStack

import numpy as np

N_CORES = 8
P = 128
PAD_VAL = -80.0

# canonical slot lengths: exact for 2..24, coarser for the rare tail
_CANON_BASE = list(range(2, 25)) + [26, 28, 30, 32, 36, 40, 44, 48, 56, 64, 80, 96, 128]


def _canon_lengths(max_len):
    canon = list(_CANON_BASE)
    while canon[-1] < max_len:
        canon.append(canon[-1] * 2)
    return np.asarray(canon, dtype=np.int64)


def _plan_buckets(index, num_segments):
    """Placement plan: maps every element to (core, flat offset) in the padded
    per-core [128, W_total] layout."""
    S = int(num_segments)
    idx = np.asarray(index).astype(np.int64)
    L = np.bincount(idx, minlength=S)
    starts = np.zeros(S + 1, dtype=np.int64)
    np.cumsum(L, out=starts[1:])

    seg1 = np.where(L == 1)[0]
    sel = np.where(L >= 2)[0]
    plan = dict(seg1=seg1, starts=starts)
    if len(sel) == 0:
        plan.update(W_total=0, buckets=[], e_src=np.empty(0, np.int64),
                    e_coreflat=np.empty(0, np.int64))
        return plan
    Ls = L[sel]
    canon = _canon_lengths(int(Ls.max()))
    Lc = canon[np.searchsorted(canon, Ls, side="left")]

    order = np.argsort(Lc, kind="stable")
    segs_sorted = sel[order]
    Ls_sorted = Ls[order]
    Lc_sorted = Lc[order]

    uniq, ustart, ucount = np.unique(Lc_sorted, return_index=True, return_counts=True)

    buckets = []                               # (Lb, q_b, col_b)
    col = 0
    nseg = len(segs_sorted)
    seg_core = np.empty(nseg, dtype=np.int64)
    seg_col = np.empty(nseg, dtype=np.int64)
    seg_prow = np.empty(nseg, dtype=np.int64)
    for Lb, s0, n in zip(uniq, ustart, ucount):
        Lb = int(Lb); s0 = int(s0); n = int(n)
        c = -(-n // N_CORES)                   # segs per core (ceil)
        q = -(-c // P)                         # slots per partition
        j = np.arange(n)
        core = j // c
        j_loc = j - core * c
        p = j_loc // q
        t = j_loc - p * q
        seg_core[s0:s0 + n] = core
        seg_prow[s0:s0 + n] = p
        seg_col[s0:s0 + n] = col + t * Lb
        buckets.append((Lb, q, col))
        col += q * Lb
    W_total = col

    tot_el = int(Ls_sorted.sum())
    off = np.zeros(nseg + 1, dtype=np.int64)
    np.cumsum(Ls_sorted, out=off[1:])
    within = np.arange(tot_el) - np.repeat(off[:-1], Ls_sorted)
    e_src = np.repeat(starts[segs_sorted], Ls_sorted) + within
    flat = seg_prow * W_total + seg_col
    e_flat = np.repeat(flat, Ls_sorted) + within
    e_core = np.repeat(seg_core, Ls_sorted)
    plan.update(W_total=W_total, buckets=buckets, e_src=e_src,
                e_coreflat=e_core * (P * W_total) + e_flat)
    return plan


def _build_inputs(logits, plan):
    W_total = plan["W_total"]
    xin = np.full(N_CORES * P * W_total, PAD_VAL, dtype=np.float32)
    xin[plan["e_coreflat"]] = np.asarray(logits, dtype=np.float32)[plan["e_src"]]
    return xin.reshape(N_CORES, P * W_total)


def _gather_output(results_flat, plan, n):
    out = np.zeros(n, dtype=np.float32)
    out[plan["e_src"]] = results_flat.reshape(-1)[plan["e_coreflat"]]
    out[plan["starts"][plan["seg1"]]] = 0.0
    return out


def _make_groups(buckets, target=2048, cap=2560):
    """Split bucket column ranges into contiguous ~target-column groups of
    whole segment slots; each group is a list of (col, q_slice, Lb)."""
    slices = []
    for (Lb, q, col) in buckets:
        qk = max(1, target // Lb)
        t = 0
        while t < q:
            qs = min(qk, q - t)
            slices.append((col + t * Lb, qs, Lb))
            t += qs
    groups, cur, cur_cols = [], [], 0
    for s in slices:
        scols = s[1] * s[2]
        if cur and cur_cols + scols > cap:
            groups.append(cur)
            cur, cur_cols = [], 0
        cur.append(s)
        cur_cols += scols
    if cur:
        groups.append(cur)
    return groups


def _build_program(W_total, buckets, ebufs=3, target=2048, cap=2560, n_stages=2):
    """Two-stage pipeline (best measured): stage B's loads/exp/reduce overlap
    stage A's subtract/store. Loads issue on the sync HWDGE ring, stores on the
    scalar HWDGE ring (no FIFO head-of-line blocking between them). Per-stage
    Ln keeps ACT table switches to 4 total. x tiles persist per group; the
    subtract runs in place on x."""
    import concourse.bacc as bacc
    import concourse.mybir as mybir
    from concourse import tile

    F32 = mybir.dt.float32
    nc = bacc.Bacc("TRN2", target_bir_lowering=False, debug=False,
                   num_devices=N_CORES)
    xin = nc.dram_tensor("xin", [P * W_total], F32, kind="ExternalInput").ap()
    xout = nc.dram_tensor("xout", [P * W_total], F32, kind="ExternalOutput").ap()
    xin2d = xin.rearrange("(p w) -> p w", p=P)
    xout2d = xout.rearrange("(p w) -> p w", p=P)

    groups = _make_groups(buckets, target=target, cap=cap)
    Q_total = sum(qs for g in groups for (_, qs, _) in g)

    # split groups into n_stages consecutive chunks, balanced by columns
    gcols = [g[-1][0] + g[-1][1] * g[-1][2] - g[0][0] for g in groups]
    total_cols = sum(gcols)
    stages, cur, acc = [], [], 0
    for g, gc in zip(groups, gcols):
        cur.append(g)
        acc += gc
        if (acc >= total_cols * (len(stages) + 1) / n_stages - 1
                and len(stages) < n_stages - 1):
            stages.append(cur)
            cur = []
    if cur:
        stages.append(cur)

    qof, xts = {}, {}

    with tile.TileContext(nc) as tc, ExitStack() as ctx:
        xpool = ctx.enter_context(tc.tile_pool(name="x", bufs=1))
        epool = ctx.enter_context(tc.tile_pool(name="e", bufs=ebufs))
        spool = ctx.enter_context(tc.tile_pool(name="s", bufs=1))

        st = spool.tile([P, Q_total], F32, tag="s")
        ct = spool.tile([P, Q_total], F32, tag="c")
        qoff = 0
        gid = 0

        def phaseA(g):
            nonlocal qoff, gid
            g0, g1 = g[0][0], g[-1][0] + g[-1][1] * g[-1][2]
            xt = xpool.tile([P, g1 - g0], F32, tag=f"x{gid}")
            xts[gid] = xt
            nc.sync.dma_start(xt[:], xin2d[:, g0:g1])
            et = epool.tile([P, g1 - g0], F32, tag="e")
            nc.scalar.activation(et[:], xt[:], mybir.ActivationFunctionType.Exp)
            qof[gid] = qoff
            for (col, qs, Lb) in g:
                c0 = col - g0
                nc.vector.reduce_sum(
                    st[:, qoff:qoff + qs],
                    et[:, c0:c0 + qs * Lb].rearrange("p (q l) -> p q l", q=qs),
                    axis=mybir.AxisListType.X)
                qoff += qs
            gid += 1

        def phaseC(g, i):
            g0, g1 = g[0][0], g[-1][0] + g[-1][1] * g[-1][2]
            xt = xts[i]
            q = qof[i]
            for (col, qs, Lb) in g:
                c0 = col - g0
                nc.vector.tensor_sub(
                    xt[:, c0:c0 + qs * Lb].rearrange("p (q l) -> p q l", q=qs),
                    xt[:, c0:c0 + qs * Lb].rearrange("p (q l) -> p q l", q=qs),
                    ct[:, q:q + qs].unsqueeze(2).broadcast_to([P, qs, Lb]))
                q += qs
            nc.scalar.dma_start(xout2d[:, g0:g1], xt[:])

        stage_ids = []
        for si, stage in enumerate(stages):
            q0 = qoff
            ids = []
            for g in stage:
                ids.append((g, gid))
                phaseA(g)
            stage_ids.append(ids)
            nc.scalar.activation(ct[:, q0:qoff], st[:, q0:qoff],
                                 mybir.ActivationFunctionType.Ln)
            if si > 0:
                # subtract/store of the previous stage overlaps this stage's
                # compute tail and the loads already in flight
                for (g, i) in stage_ids[si - 1]:
                    phaseC(g, i)
        for (g, i) in stage_ids[-1]:
            phaseC(g, i)
    nc.compile()
    return nc


_cache = {}


def _get_program(plan):
    key = (plan["W_total"], tuple(plan["buckets"]))
    if key not in _cache:
        _cache[key] = _build_program(plan["W_total"], plan["buckets"])
    return _cache[key]


def run_on_device(nc, xin_cores, trace=False, **kw):
    from concourse.bass_utils import run_bass_kernel_spmd
    in_maps = [{"xin": xin_cores[c]} for c in range(N_CORES)]
    res = run_bass_kernel_spmd(nc, in_maps, core_ids=list(range(N_CORES)),
                               trace=trace, **kw)
    out = np.stack([res.results[c]["xout"] for c in range(N_CORES)])
    return out, res


def kernel(logits, index, num_segments):
    logits = np.asarray(logits)
    n = logits.shape[0]
    plan = _plan_buckets(index, num_segments)
    if plan["W_total"] == 0:
        out = np.zeros(n, dtype=np.float32)
        out[plan["starts"][plan["seg1"]]] = 0.0
        return out
    xin = _build_inputs(logits, plan)
    nc = _get_program(plan)
    out_flat, _ = run_on_device(nc, xin)
    return _gather_output(out_flat, plan, n)



# revision 2
# speedup vs baseline: 1.1153x; 1.1153x over previous
"""Grouped categorical log-softmax (segment logsumexp) on 8 Trainium2 cores.

The index is sorted, so each segment is a contiguous run. Host-side we bucket
segments by canonical length, shard every bucket evenly across the 8 cores,
and lay each core's data out as a dense [128, W] fp16 matrix. Inside a bucket
slice the layout is ELEMENT-major: a slice of q slots with length L occupies
L*q contiguous columns as L row-planes of q columns each; element j of slot t
sits at col0 + j*q + t. Short segments are padded with -88 (exp == 0).

Device pipeline per column-group (software-pipelined across groups):
  DMA-in (sync ring, fp16)
  -> exp on ScalarE (fp16 -> fp16)
  -> one/two halving folds on VectorE (dense fp16 tensor_add, 2x mode)
  -> per-slice PSUM accumulation of the remaining row-planes on TensorE
     (identity-weight matmuls; free-axis segment reduce on the idle PE)
  -> Ln on ScalarE (PSUM fp32 -> SBUF fp16)  [exp+ln share one ACT table set]
  -> broadcast subtract on VectorE: x[:, j*q+t] -= lse[:, t]
     (second operand has innermost step 1 -> DVE 2x_1p mode)
  -> DMA-out (gpsimd ring, fp16).

out = x - log(sum(exp(x))) is mathematically identical to the reference's
max-normalized form; with standard-normal logits fp32/fp16 exp is nowhere
near overflow, and the end-to-end fp16 rounding gives ~1e-3 max abs error
against the fp32 reference (denominator absmax ~9), far inside the 2e-2 gate.

Length-1 segments are exactly 0 in the reference and are filled on the host.
"""
from contextlib import ExitStack

import numpy as np

N_CORES = 8
P = 128
PAD = -88.0

GW_TARGET = 2560   # target group width in columns (pipeline unit)
QG_MAX = 448       # max slots per group (PSUM bank budget, <= 512)
X_BUFS = 4
E_BUFS = 3


def _canon_len(L):
    """Canonical slot lengths: exact 2..9, even 10..24, mult-4 to 48,
    mult-8 to 96, mult-16 beyond (keeps fold chains even and buckets few)."""
    L = np.asarray(L, dtype=np.int64)
    return np.where(L <= 9, L,
           np.where(L <= 24, L + (L & 1),
           np.where(L <= 48, -(-L // 4) * 4,
           np.where(L <= 96, -(-L // 8) * 8,
                    -(-L // 16) * 16))))


def _n_folds(L):
    if L < 28:
        return 1
    if L < 56:
        return 2
    if L < 112:
        return 3
    return 4


def _plan_buckets(index, num_segments):
    S = int(num_segments)
    idx = np.asarray(index).astype(np.int64)
    Lfull = np.bincount(idx, minlength=S)
    starts = np.zeros(S + 1, dtype=np.int64)
    np.cumsum(Lfull, out=starts[1:])

    seg1 = np.where(Lfull == 1)[0]
    sel = np.where(Lfull >= 2)[0]
    plan = dict(seg1=seg1, starts=starts)
    if len(sel) == 0:
        plan.update(W=0, Q=0, groups=[], e_src=np.empty(0, np.int64),
                    e_coreflat=np.empty(0, np.int64))
        return plan

    Ls = Lfull[sel]
    Lc = _canon_len(Ls)
    order = np.argsort(Lc, kind="stable")
    segs = sel[order]
    Ls_o = Ls[order]
    Lc_o = Lc[order]
    uniq, ustart, ucount = np.unique(Lc_o, return_index=True,
                                     return_counts=True)

    # per bucket: per-core seg count c_b and (even) slots-per-partition q_b
    binfo = []
    for Lb, n in zip(uniq, ucount):
        c = -(-int(n) // N_CORES)
        q = -(-c // P)
        q += q & 1
        binfo.append([int(Lb), int(c), int(q)])

    # pack bucket slot-ranges into column groups (split at even slot counts)
    groups = []          # list of list of (L, qs, col, qoff, t0)
    bslices = {}         # bucket idx -> list of (t0, qs, col, qoff)
    cur, cur_cols, cur_q = [], 0, 0
    col = qoff = 0
    for b, (Lb, c, qb) in enumerate(binfo):
        bslices[b] = []
        t0 = 0
        while t0 < qb:
            qs = min(qb - t0, (GW_TARGET - cur_cols) // Lb,
                     QG_MAX - cur_q, 512)
            qs -= qs & 1
            if qs < 2:
                if cur:
                    groups.append(cur)
                cur, cur_cols, cur_q = [], 0, 0
                continue
            cur.append((Lb, qs, col, qoff, t0))
            bslices[b].append((t0, qs, col, qoff))
            col += Lb * qs
            qoff += qs
            cur_cols += Lb * qs
            cur_q += qs
            t0 += qs
    if cur:
        groups.append(cur)
    W, Q = col, qoff

    # per-segment placement: core, partition p, slot t -> slice -> column
    nseg = len(segs)
    seg_base = np.empty(nseg, dtype=np.int64)   # core*(P*W) + p*W + col of elem 0
    seg_qs = np.empty(nseg, dtype=np.int64)     # column stride between elements
    for b, (Lb, c, qb) in enumerate(binfo):
        s0, n = int(ustart[b]), int(ucount[b])
        j = np.arange(n)
        core = j // c
        j_loc = j - core * c
        p = j_loc // qb
        t = j_loc - p * qb
        bt0 = np.array([s[0] for s in bslices[b]], dtype=np.int64)
        bqs = np.array([s[1] for s in bslices[b]], dtype=np.int64)
        bcol = np.array([s[2] for s in bslices[b]], dtype=np.int64)
        k = np.searchsorted(bt0, t, side="right") - 1
        seg_base[s0:s0 + n] = core * (P * W) + p * W + bcol[k] + (t - bt0[k])
        seg_qs[s0:s0 + n] = bqs[k]

    tot = int(Ls_o.sum())
    off = np.zeros(nseg + 1, dtype=np.int64)
    np.cumsum(Ls_o, out=off[1:])
    within = np.arange(tot, dtype=np.int64) - np.repeat(off[:-1], Ls_o)
    e_src = np.repeat(starts[segs], Ls_o) + within
    e_coreflat = np.repeat(seg_base, Ls_o) + within * np.repeat(seg_qs, Ls_o)
    plan.update(W=W, Q=Q, groups=groups, e_src=e_src, e_coreflat=e_coreflat)
    return plan


def _build_inputs(logits, plan):
    W = plan["W"]
    xin = np.full(N_CORES * P * W, PAD, dtype=np.float16)
    xin[plan["e_coreflat"]] = np.asarray(logits, dtype=np.float16)[plan["e_src"]]
    return xin.reshape(N_CORES, P * W)


def _gather_output(results_flat, plan, n):
    out = np.zeros(n, dtype=np.float32)
    out[plan["e_src"]] = results_flat.reshape(-1)[plan["e_coreflat"]].astype(np.float32)
    out[plan["starts"][plan["seg1"]]] = 0.0
    return out


def _build_program(W, Q, groups):
    import concourse.bacc as bacc
    import concourse.mybir as mybir
    from concourse import tile

    F16 = mybir.dt.float16
    F32 = mybir.dt.float32
    Exp = mybir.ActivationFunctionType.Exp
    Ln = mybir.ActivationFunctionType.Ln

    nc = bacc.Bacc("TRN2", target_bir_lowering=False, debug=False,
                   num_devices=N_CORES)
    xin = nc.dram_tensor("xin", [P * W], F16, kind="ExternalInput").ap()
    xout = nc.dram_tensor("xout", [P * W], F16, kind="ExternalOutput").ap()
    ident = nc.dram_tensor("ident", [P * P], F16, kind="ExternalInput").ap()
    xin2d = xin.rearrange("(p w) -> p w", p=P)
    xout2d = xout.rearrange("(p w) -> p w", p=P)
    id2d = ident.rearrange("(p w) -> p w", p=P)

    def gspan(g):
        g0 = g[0][2]
        g1 = g[-1][2] + g[-1][0] * g[-1][1]
        q0 = g[0][3]
        q1 = g[-1][3] + g[-1][1]
        return g0, g1, q0, q1

    GW = max(gspan(g)[1] - gspan(g)[0] for g in groups)
    QG = max(gspan(g)[3] - gspan(g)[2] for g in groups)

    with tile.TileContext(nc) as tc, ExitStack() as ctx:
        xp = ctx.enter_context(tc.tile_pool(name="x", bufs=X_BUFS))
        ep = ctx.enter_context(tc.tile_pool(name="e", bufs=E_BUFS))
        cp = ctx.enter_context(tc.tile_pool(name="c", bufs=1))
        wp = ctx.enter_context(tc.tile_pool(name="w", bufs=1))
        pp = ctx.enter_context(tc.tile_pool(name="ps", bufs=2, space="PSUM"))

        # preload the one ACT table set containing BOTH exp and ln so the
        # compiler never inserts mid-kernel table switches (~2.7us each)
        try:
            from concourse.hw_specs import get_activation_tables
            tnames = [t for t, _ in get_activation_tables(nc.m.arch).items()]
            combo = tnames.index("natural_log_exp_and_others")
            nc.scalar.add_instruction(mybir.InstLoadActFuncSet(
                name=nc.get_next_instruction_name(),
                act_func_set_id=combo, ins=[], outs=[]))
        except Exception:
            pass

        wt = wp.tile([P, P], F16, tag="w")
        nc.sync.dma_start(wt[:], id2d[:, :])
        ct = cp.tile([P, Q], F16, tag="ct")

        def folds_and_mms(g, et, ps):
            g0, _, q0, _ = gspan(g)
            for (L, qs, coll, qof, _) in g:
                r0 = coll - g0
                cur = L
                for _ in range(_n_folds(L)):
                    if cur < 2:
                        break
                    h = cur // 2
                    nc.vector.tensor_add(
                        et[:, r0:r0 + h * qs],
                        et[:, r0:r0 + h * qs],
                        et[:, r0 + (cur - h) * qs:r0 + cur * qs])
                    cur -= h
            for (L, qs, coll, qof, _) in g:
                r0 = coll - g0
                cur = L
                for _ in range(_n_folds(L)):
                    if cur < 2:
                        break
                    cur -= cur // 2
                ql = qof - q0
                for j in range(cur):
                    nc.tensor.matmul(
                        ps[:, ql:ql + qs],
                        wt[:],
                        et[:, r0 + j * qs:r0 + (j + 1) * qs],
                        start=(j == 0), stop=(j == cur - 1))

        def subs_and_store(g, xt):
            g0, g1, _, _ = gspan(g)
            for (L, qs, coll, qof, _) in g:
                r0 = coll - g0
                x3 = xt[:, r0:r0 + L * qs].rearrange("p (l q) -> p l q", l=L)
                cb = ct[:, qof:qof + qs].unsqueeze(1).broadcast_to([P, L, qs])
                nc.vector.tensor_sub(x3, x3, cb)
            nc.gpsimd.dma_start(xout2d[:, g0:g1], xt[:, :g1 - g0])

        prev = None
        for g in groups:
            g0, g1, q0, q1 = gspan(g)
            xt = xp.tile([P, GW], F16, tag="x")
            nc.sync.dma_start(xt[:, :g1 - g0], xin2d[:, g0:g1])
            et = ep.tile([P, GW], F16, tag="e")
            nc.scalar.activation(et[:, :g1 - g0], xt[:, :g1 - g0], Exp)
            if prev is not None:
                pg, pxt, pps = prev
                p0, p1, pq0, pq1 = gspan(pg)
                nc.scalar.activation(ct[:, pq0:pq1], pps[:, :pq1 - pq0], Ln)
            ps = pp.tile([P, QG], F32, tag="ps")
            folds_and_mms(g, et, ps)
            if prev is not None:
                subs_and_store(prev[0], prev[1])
            prev = (g, xt, ps)
        pg, pxt, pps = prev
        p0, p1, pq0, pq1 = gspan(pg)
        nc.scalar.activation(ct[:, pq0:pq1], pps[:, :pq1 - pq0], Ln)
        subs_and_store(pg, pxt)

    nc.compile()
    return nc


_cache = {}


def _get_program(plan):
    key = (plan["W"], plan["Q"],
           tuple(tuple(s) for g in plan["groups"] for s in g),
           tuple(len(g) for g in plan["groups"]))
    if key not in _cache:
        _cache[key] = _build_program(plan["W"], plan["Q"], plan["groups"])
    return _cache[key]


def run_on_device(nc, xin_cores, trace=False, **kw):
    from concourse.bass_utils import run_bass_kernel_spmd
    ident = np.eye(P, dtype=np.float16).reshape(-1)
    in_maps = [{"xin": xin_cores[c], "ident": ident} for c in range(N_CORES)]
    res = run_bass_kernel_spmd(nc, in_maps, core_ids=list(range(N_CORES)),
                               trace=trace, **kw)
    out = np.stack([res.results[c]["xout"] for c in range(N_CORES)])
    return out, res


def kernel(logits, index, num_segments):
    logits = np.asarray(logits)
    n = logits.shape[0]
    plan = _plan_buckets(index, num_segments)
    if plan["W"] == 0:
        out = np.zeros(n, dtype=np.float32)
        out[plan["starts"][plan["seg1"]]] = 0.0
        return out
    xin = _build_inputs(logits, plan)
    nc = _get_program(plan)
    out_flat, _ = run_on_device(nc, xin)
    return _gather_output(out_flat, plan, n)


# revision 8
# speedup vs baseline: 1.4152x; 1.2690x over previous
"""Grouped categorical log-softmax (segment logsumexp) on 8 Trainium2 cores.

The index is sorted, so each segment is a contiguous run. Host-side we bucket
segments by canonical length, shard every bucket evenly across the 8 cores,
and lay each core's data out as a dense [128, W] fp16 matrix. Inside a bucket
slice the layout is ELEMENT-major: a slice of q slots with length L occupies
L*q contiguous columns as L row-planes of q columns each; element j of slot t
sits at col0 + j*q + t. Short segments are padded with -88 (exp == 0).

Device pipeline per column-group (software-pipelined across groups):
  DMA-in (sync ring, fp16)
  -> exp on ScalarE (fp16 -> fp16)
  -> one/two halving folds on VectorE (dense fp16 tensor_add, 2x mode)
  -> per-slice PSUM accumulation of the remaining row-planes on TensorE
     (identity-weight matmuls; free-axis segment reduce on the idle PE)
  -> Ln on ScalarE (PSUM fp32 -> SBUF fp16)  [exp+ln share one ACT table set]
  -> broadcast subtract on VectorE: x[:, j*q+t] -= lse[:, t]
     (second operand has innermost step 1 -> DVE 2x_1p mode)
  -> DMA-out (gpsimd ring, fp16).

out = x - log(sum(exp(x))) is mathematically identical to the reference's
max-normalized form; with standard-normal logits fp32/fp16 exp is nowhere
near overflow, and the end-to-end fp16 rounding gives ~1e-3 max abs error
against the fp32 reference (denominator absmax ~9), far inside the 2e-2 gate.

Length-1 segments are exactly 0 in the reference and are filled on the host.
"""
from contextlib import ExitStack

import numpy as np

N_CORES = 8
P = 128
PAD = -88.0

GW_RAMP = [512, 1024, 2048]  # widths of the first groups (fast pipeline fill)
GW_TARGET = 4096   # steady-state group width in columns
QG_MAX = 448       # max slots per group (PSUM bank budget, <= 512)
E_BUFS = 3
FOLD_MIN_COLS = 512  # skip DVE pre-fold when a slice has less data than this


def _canon_len(L):
    """Canonical slot lengths: exact 2..9, even 10..24, mult-4 to 48,
    mult-8 to 96, mult-16 beyond (keeps fold chains even and buckets few)."""
    L = np.asarray(L, dtype=np.int64)
    return np.where(L <= 9, L,
           np.where(L <= 24, L + (L & 1),
           np.where(L <= 48, -(-L // 4) * 4,
           np.where(L <= 96, -(-L // 8) * 8,
                    -(-L // 16) * 16))))


def _n_folds(L):
    if L < 28:
        return 1
    if L < 56:
        return 2
    if L < 112:
        return 3
    return 4


def _plan_buckets(index, num_segments):
    S = int(num_segments)
    idx = np.asarray(index).astype(np.int64)
    Lfull = np.bincount(idx, minlength=S)
    starts = np.zeros(S + 1, dtype=np.int64)
    np.cumsum(Lfull, out=starts[1:])

    seg1 = np.where(Lfull == 1)[0]
    sel = np.where(Lfull >= 2)[0]
    plan = dict(seg1=seg1, starts=starts)
    if len(sel) == 0:
        plan.update(W=0, Q=0, groups=[], e_src=np.empty(0, np.int64),
                    e_coreflat=np.empty(0, np.int64))
        return plan

    Ls = Lfull[sel]
    Lc = _canon_len(Ls)
    order = np.argsort(Lc, kind="stable")
    segs = sel[order]
    Ls_o = Ls[order]
    Lc_o = Lc[order]
    uniq, ustart, ucount = np.unique(Lc_o, return_index=True,
                                     return_counts=True)

    # per bucket: per-core seg count c_b and (even) slots-per-partition q_b
    binfo = []
    for Lb, n in zip(uniq, ucount):
        c = -(-int(n) // N_CORES)
        q = -(-c // P)
        q += q & 1
        binfo.append([int(Lb), int(c), int(q)])

    # pack bucket slot-ranges into column groups (split at even slot counts);
    # the first groups are small so the compute pipeline fills quickly
    groups = []          # list of list of (L, qs, col, qoff, t0)
    bslices = {}         # bucket idx -> list of (t0, qs, col, qoff)
    cur, cur_cols, cur_q = [], 0, 0
    col = qoff = 0

    def gw_cap():
        gi = len(groups)
        return GW_RAMP[gi] if gi < len(GW_RAMP) else GW_TARGET

    for b, (Lb, c, qb) in enumerate(binfo):
        bslices[b] = []
        t0 = 0
        while t0 < qb:
            qs = min(qb - t0, (gw_cap() - cur_cols) // Lb,
                     QG_MAX - cur_q, 512)
            qs -= qs & 1
            if qs < 2:
                if cur:
                    groups.append(cur)
                cur, cur_cols, cur_q = [], 0, 0
                continue
            cur.append((Lb, qs, col, qoff, t0))
            bslices[b].append((t0, qs, col, qoff))
            col += Lb * qs
            qoff += qs
            cur_cols += Lb * qs
            cur_q += qs
            t0 += qs
    if cur:
        groups.append(cur)
    W, Q = col, qoff

    # per-segment placement: core, partition p, slot t -> slice -> column
    nseg = len(segs)
    seg_base = np.empty(nseg, dtype=np.int64)   # core*(P*W) + p*W + col of elem 0
    seg_qs = np.empty(nseg, dtype=np.int64)     # column stride between elements
    for b, (Lb, c, qb) in enumerate(binfo):
        s0, n = int(ustart[b]), int(ucount[b])
        j = np.arange(n)
        core = j // c
        j_loc = j - core * c
        p = j_loc // qb
        t = j_loc - p * qb
        bt0 = np.array([s[0] for s in bslices[b]], dtype=np.int64)
        bqs = np.array([s[1] for s in bslices[b]], dtype=np.int64)
        bcol = np.array([s[2] for s in bslices[b]], dtype=np.int64)
        k = np.searchsorted(bt0, t, side="right") - 1
        seg_base[s0:s0 + n] = core * (P * W) + p * W + bcol[k] + (t - bt0[k])
        seg_qs[s0:s0 + n] = bqs[k]

    tot = int(Ls_o.sum())
    off = np.zeros(nseg + 1, dtype=np.int64)
    np.cumsum(Ls_o, out=off[1:])
    within = np.arange(tot, dtype=np.int64) - np.repeat(off[:-1], Ls_o)
    e_src = np.repeat(starts[segs], Ls_o) + within
    e_coreflat = np.repeat(seg_base, Ls_o) + within * np.repeat(seg_qs, Ls_o)
    plan.update(W=W, Q=Q, groups=groups, e_src=e_src, e_coreflat=e_coreflat)
    return plan


def _build_inputs(logits, plan):
    W = plan["W"]
    xin = np.full(N_CORES * P * W, PAD, dtype=np.float16)
    xin[plan["e_coreflat"]] = np.asarray(logits, dtype=np.float16)[plan["e_src"]]
    return xin.reshape(N_CORES, P * W)


def _gather_output(results_flat, plan, n):
    out = np.zeros(n, dtype=np.float32)
    out[plan["e_src"]] = results_flat.reshape(-1)[plan["e_coreflat"]].astype(np.float32)
    out[plan["starts"][plan["seg1"]]] = 0.0
    return out


def _build_program(W, Q, groups):
    import concourse.bacc as bacc
    import concourse.mybir as mybir
    from concourse import tile

    F16 = mybir.dt.float16
    F32 = mybir.dt.float32
    Exp = mybir.ActivationFunctionType.Exp
    Ln = mybir.ActivationFunctionType.Ln

    nc = bacc.Bacc("TRN2", target_bir_lowering=False, debug=False,
                   num_devices=N_CORES)
    xin = nc.dram_tensor("xin", [P * W], F16, kind="ExternalInput").ap()
    xout = nc.dram_tensor("xout", [P * W], F16, kind="ExternalOutput").ap()
    ident = nc.dram_tensor("ident", [P * P], F16, kind="ExternalInput").ap()
    xin2d = xin.rearrange("(p w) -> p w", p=P)
    xout2d = xout.rearrange("(p w) -> p w", p=P)
    id2d = ident.rearrange("(p w) -> p w", p=P)

    def gspan(g):
        g0 = g[0][2]
        g1 = g[-1][2] + g[-1][0] * g[-1][1]
        q0 = g[0][3]
        q1 = g[-1][3] + g[-1][1]
        return g0, g1, q0, q1

    GW = max(gspan(g)[1] - gspan(g)[0] for g in groups)
    QG = max(gspan(g)[3] - gspan(g)[2] for g in groups)

    with tile.TileContext(nc) as tc, ExitStack() as ctx:
        xp = ctx.enter_context(tc.tile_pool(name="x", bufs=1))
        ep = ctx.enter_context(tc.tile_pool(name="e", bufs=E_BUFS))
        cp = ctx.enter_context(tc.tile_pool(name="c", bufs=1))
        wp = ctx.enter_context(tc.tile_pool(name="w", bufs=1))
        pp = ctx.enter_context(tc.tile_pool(name="ps", bufs=2, space="PSUM"))

        # preload the one ACT table set containing BOTH exp and ln so the
        # compiler never inserts mid-kernel table switches (~2.7us each)
        try:
            from concourse.hw_specs import get_activation_tables
            tnames = [t for t, _ in get_activation_tables(nc.m.arch).items()]
            combo = tnames.index("natural_log_exp_and_others")
            nc.scalar.add_instruction(mybir.InstLoadActFuncSet(
                name=nc.get_next_instruction_name(),
                act_func_set_id=combo, ins=[], outs=[]))
        except Exception:
            pass

        wt = wp.tile([P, P], F16, tag="w")
        nc.sync.dma_start(wt[:], id2d[:, :])
        ct = cp.tile([P, Q], F16, tag="ct")

        def slice_folds(L, qs):
            return _n_folds(L) if L * qs >= FOLD_MIN_COLS else 0

        def folds_and_mms(g, et, ps):
            g0, _, q0, _ = gspan(g)
            for (L, qs, coll, qof, _) in g:
                r0 = coll - g0
                cur = L
                for _ in range(slice_folds(L, qs)):
                    if cur < 2:
                        break
                    h = cur // 2
                    nc.vector.tensor_add(
                        et[:, r0:r0 + h * qs],
                        et[:, r0:r0 + h * qs],
                        et[:, r0 + (cur - h) * qs:r0 + cur * qs])
                    cur -= h
            for (L, qs, coll, qof, _) in g:
                r0 = coll - g0
                cur = L
                for _ in range(slice_folds(L, qs)):
                    if cur < 2:
                        break
                    cur -= cur // 2
                ql = qof - q0
                for j in range(cur):
                    nc.tensor.matmul(
                        ps[:, ql:ql + qs],
                        wt[:],
                        et[:, r0 + j * qs:r0 + (j + 1) * qs],
                        start=(j == 0), stop=(j == cur - 1))

        def subs_and_store(g, xt):
            g0, g1, _, _ = gspan(g)
            for (L, qs, coll, qof, _) in g:
                r0 = coll - g0
                x3 = xt[:, r0:r0 + L * qs].rearrange("p (l q) -> p l q", l=L)
                cb = ct[:, qof:qof + qs].unsqueeze(1).broadcast_to([P, L, qs])
                nc.vector.tensor_sub(x3, x3, cb)
            nc.gpsimd.dma_start(xout2d[:, g0:g1], xt[:, :g1 - g0])

        # all input tiles stay resident; issue every load upfront so the
        # load stream is never throttled by buffer recycling
        xts = []
        for gi, g in enumerate(groups):
            g0, g1, _, _ = gspan(g)
            xt = xp.tile([P, g1 - g0], F16, tag=f"x{gi}")
            nc.sync.dma_start(xt[:], xin2d[:, g0:g1])
            xts.append(xt)

        prev = None
        for gi, g in enumerate(groups):
            g0, g1, q0, q1 = gspan(g)
            xt = xts[gi]
            et = ep.tile([P, GW], F16, tag="e")
            nc.scalar.activation(et[:, :g1 - g0], xt[:], Exp)
            if prev is not None:
                pg, pxt, pps = prev
                p0, p1, pq0, pq1 = gspan(pg)
                nc.scalar.activation(ct[:, pq0:pq1], pps[:, :pq1 - pq0], Ln)
            ps = pp.tile([P, QG], F32, tag="ps")
            folds_and_mms(g, et, ps)
            if prev is not None:
                subs_and_store(prev[0], prev[1])
            prev = (g, xt, ps)
        pg, pxt, pps = prev
        p0, p1, pq0, pq1 = gspan(pg)
        nc.scalar.activation(ct[:, pq0:pq1], pps[:, :pq1 - pq0], Ln)
        subs_and_store(pg, pxt)

    nc.compile()
    return nc


_cache = {}


def _get_program(plan):
    key = (plan["W"], plan["Q"],
           tuple(tuple(s) for g in plan["groups"] for s in g),
           tuple(len(g) for g in plan["groups"]))
    if key not in _cache:
        _cache[key] = _build_program(plan["W"], plan["Q"], plan["groups"])
    return _cache[key]


def run_on_device(nc, xin_cores, trace=False, **kw):
    from concourse.bass_utils import run_bass_kernel_spmd
    ident = np.eye(P, dtype=np.float16).reshape(-1)
    in_maps = [{"xin": xin_cores[c], "ident": ident} for c in range(N_CORES)]
    res = run_bass_kernel_spmd(nc, in_maps, core_ids=list(range(N_CORES)),
                               trace=trace, **kw)
    out = np.stack([res.results[c]["xout"] for c in range(N_CORES)])
    return out, res


def kernel(logits, index, num_segments):
    logits = np.asarray(logits)
    n = logits.shape[0]
    plan = _plan_buckets(index, num_segments)
    if plan["W"] == 0:
        out = np.zeros(n, dtype=np.float32)
        out[plan["starts"][plan["seg1"]]] = 0.0
        return out
    xin = _build_inputs(logits, plan)
    nc = _get_program(plan)
    out_flat, _ = run_on_device(nc, xin)
    return _gather_output(out_flat, plan, n)
